# revision 1
# baseline (speedup 1.0000x reference)
"""Confidence-histogram (ECE bin stats) Trainium2 Bass kernel.

Full-input contract: kernel(logits[1M,128] f32, labels[1M] int) -> [15,2] f32.

Math: conf = max(softmax(x)) = exp(max(x)) / sum(exp(x)); prediction is
correct iff x[label] == max(x) (ties are measure-zero for randn inputs).
Binning is done with cumulative threshold counts in log space:
    t = max(x) - ln(sum(exp(x)));   conf >= b/15  <=>  t >= ln(b/15)
Each core computes, for b in 1..14, ct_b = #{t >= theta_b} and
cc_b = #{t >= theta_b and correct}, plus cc_0 = total correct, as
per-partition partial sums. Host diffs the cumulative counts into the
[15,2] (correct, incorrect) table.

Sharding: data-parallel over N across 8 cores; each 125k-sample shard is
padded to 128*992 rows laid out partition-major so every DMA descriptor
reads 8KB contiguous HBM. Pad rows are [1,0,...,0] with label-logit 0:
conf=e/(e+127)~0.021 < 1/15 so they never cross any threshold, and they
are never counted correct, making them invisible to the output.
"""

import numpy as np

import concourse.bass as bass
import concourse.bacc as bacc
import concourse.tile as tile
from concourse import mybir
from concourse.bass_utils import run_bass_kernel_spmd

N_BINS = 15
C = 128
N_CORES = 8
P = 128          # SBUF partitions
Q = 992          # samples per partition per core (padded)
N_PAD = P * Q    # 126976 padded samples per core
CH = 16          # 128-sample slices per chunk (1MB DMA)
N_CHUNKS = Q // CH  # 62

_F32 = mybir.dt.float32


def _build_bass(q: int = Q, ch: int = CH) -> bass.Bass:
    n_pad = P * q
    n_chunks = q // ch
    nc = bacc.Bacc(None, target_bir_lowering=False)
    lg = nc.dram_tensor("logits", [n_pad, C], _F32, kind="ExternalInput")
    xl = nc.dram_tensor("xl", [P, q], _F32, kind="ExternalInput")
    out = nc.dram_tensor("out", [P, 32], _F32, kind="ExternalOutput")

    # partition-major view: sample (p, q) lives at HBM row p*Q + q
    lgv = lg[:].rearrange("(p q) c -> p q c", p=P)

    # thresholds ln(b/15) computed from the same f32 linspace the reference uses
    lowers = np.linspace(0.0, 1.0, N_BINS + 1, dtype=np.float32)[:-1]
    thetas = [float(np.log(np.float64(lowers[b]))) for b in range(1, N_BINS)]

    with tile.TileContext(nc) as tc:
        with (
            tc.tile_pool(name="xin", bufs=3) as xpool,
            tc.tile_pool(name="eexp", bufs=3) as epool,
            tc.tile_pool(name="jnk", bufs=2) as jpool,
            tc.tile_pool(name="wide", bufs=1) as wide,
        ):
            me = wide.tile([P, q], _F32)     # per-sample max(exp(x)) = exp(max(x))
            sr = wide.tile([P, q], _F32)     # per-sample sum(exp(x))
            xlt = wide.tile([P, q], _F32)    # per-sample x[label]
            tt = wide.tile([P, q], _F32)     # t = ln(me) - ln(sr)
            accv = wide.tile([P, q], _F32)   # correctness 0/1
            mask = wide.tile([P, q], _F32)   # threshold mask scratch
            junk = wide.tile([P, q], _F32)   # scratch
            out_t = wide.tile([P, 32], _F32)

            nc.vector.memset(out_t[:], 0.0)
            nc.sync.dma_start(out=xlt[:], in_=xl[:])

            for j in range(n_chunks):
                xt = xpool.tile([P, ch, C], _F32)
                nc.sync.dma_start(out=xt[:], in_=lgv[:, j * ch : (j + 1) * ch, :])
                et = epool.tile([P, ch, C], _F32)
                # first m slices: ACT computes exp AND the per-sample sum via
                # its accumulator, offloading those sums from the DVE; the
                # rest are exp'd in one bulk op and summed on the DVE.
                m = min(4, ch)  # ACT-offloaded sums per chunk (cost-model optimum)
                for k in range(m):
                    col = j * ch + k
                    nc.scalar.activation(
                        out=et[:, k, :], in_=xt[:, k, :],
                        func=mybir.ActivationFunctionType.Exp,
                        accum_out=sr[:, col : col + 1],
                    )
                if m < ch:
                    nc.scalar.activation(
                        out=et[:, m:ch, :], in_=xt[:, m:ch, :],
                        func=mybir.ActivationFunctionType.Exp,
                    )
                jt = jpool.tile([P, ch, C], _F32)
                # per-slice tensor_scalar+accum runs in 2x DVE mode (vs 1x for
                # tensor_reduce): op1 is the accumulator's reduce op
                for k in range(ch):
                    col = j * ch + k
                    nc.vector.tensor_scalar(
                        jt[:, k, :], et[:, k, :], 0.0, None, mybir.AluOpType.add,
                        op1=mybir.AluOpType.max, accum_out=me[:, col : col + 1],
                    )
                    if k >= m:
                        nc.vector.tensor_scalar(
                            jt[:, k, :], et[:, k, :], 1.0, None, mybir.AluOpType.mult,
                            op1=mybir.AluOpType.add, accum_out=sr[:, col : col + 1],
                        )

            # ---- per-sample epilogue on [128, Q] wide tiles ----
            # t = ln(max e) - ln(sum e)  (log-confidence)
            nc.scalar.activation(
                out=tt[:], in_=me[:], func=mybir.ActivationFunctionType.Ln
            )
            nc.scalar.activation(
                out=junk[:], in_=sr[:], func=mybir.ActivationFunctionType.Ln
            )
            nc.vector.tensor_tensor(tt[:], tt[:], junk[:], mybir.AluOpType.subtract)
            # acc: exp(x[label]) == max(exp(x)), with exp computed on-device so
            # rounding matches the per-class exps exactly
            nc.scalar.activation(
                out=mask[:], in_=xlt[:], func=mybir.ActivationFunctionType.Exp
            )
            nc.vector.tensor_tensor(accv[:], mask[:], me[:], mybir.AluOpType.is_equal)
            # col 16: total correct
            nc.vector.tensor_scalar(
                junk[:], accv[:], 1.0, None, mybir.AluOpType.mult,
                op1=mybir.AluOpType.add, accum_out=out_t[:, 16:17],
            )
            for b in range(1, N_BINS):
                # col b: ct_b = sum(t >= theta_b); op1 is the accum reduce op
                nc.vector.tensor_scalar(
                    mask[:], tt[:], thetas[b - 1], None, mybir.AluOpType.is_ge,
                    op1=mybir.AluOpType.add,
                    accum_out=out_t[:, b : b + 1],
                )
                # col 16+b: cc_b = sum((t >= theta_b) * acc) in one fused op
                nc.vector.scalar_tensor_tensor(
                    out=junk[:], in0=tt[:], scalar=thetas[b - 1], in1=accv[:],
                    op0=mybir.AluOpType.is_ge, op1=mybir.AluOpType.mult,
                    accum_out=out_t[:, 16 + b : 17 + b],
                )
            nc.sync.dma_start(out=out[:], in_=out_t[:])
    nc.compile()
    return nc


_NC_CACHE = None


def _get_nc() -> bass.Bass:
    global _NC_CACHE
    if _NC_CACHE is None:
        _NC_CACHE = _build_bass()
    return _NC_CACHE


def make_in_maps(logits: np.ndarray, labels: np.ndarray):
    """Shard + pad full inputs into per-core input maps."""
    n = logits.shape[0]
    n_per = n // N_CORES
    assert n_per * N_CORES == n and n_per <= N_PAD
    idx = np.arange(n, dtype=np.int64)
    xl_full = logits[idx, labels.astype(np.int64)].astype(np.float32)

    pad_row = np.zeros(C, np.float32)
    pad_row[0] = 1.0
    in_maps = []
    for s in range(N_CORES):
        lo = s * n_per
        shard = np.empty((N_PAD, C), np.float32)
        shard[:n_per] = logits[lo : lo + n_per]
        shard[n_per:] = pad_row  # conf ~0.021 (bin 0), never correct
        xls = np.zeros(N_PAD, np.float32)
        xls[:n_per] = xl_full[lo : lo + n_per]
        in_maps.append({"logits": shard, "xl": xls.reshape(P, Q)})
    return in_maps


def combine_outputs(results, n: int) -> np.ndarray:
    """Fold per-core [128,32] partials into the [15,2] bin_stats table."""
    ct = np.zeros(N_BINS + 1, np.float64)  # cumulative totals, index b
    cc = np.zeros(N_BINS + 1, np.float64)  # cumulative corrects
    ct[0] = float(n)
    for r in results:
        o = np.asarray(r["out"], np.float64)
        colsum = o.sum(axis=0)
        ct[1:N_BINS] += colsum[1:N_BINS]
        cc[0] += colsum[16]
        cc[1:N_BINS] += colsum[17 : 16 + N_BINS]
    total = ct[:-1] - ct[1:]
    correct = cc[:-1] - cc[1:]
    return np.stack([correct, total - correct], axis=1).astype(np.float32)


def kernel(logits, labels) -> np.ndarray:
    logits = np.asarray(logits, dtype=np.float32)
    labels = np.asarray(labels)
    n = logits.shape[0]
    in_maps = make_in_maps(logits, labels)
    res = run_bass_kernel_spmd(_get_nc(), in_maps, core_ids=list(range(N_CORES)))
    return combine_outputs(res.results, n)



# revision 2
# speedup vs baseline: 1.7257x; 1.7257x over previous
"""Confidence-histogram (ECE bin stats) Trainium2 Bass kernel.

Full-input contract: kernel(logits[1M,128] f32, labels[1M] int) -> [15,2] f32.

Math: conf = max(softmax(x)) = exp(max(x)) / sum(exp(x)); prediction is
correct iff x[label] == max(x) (ties are measure-zero for randn inputs).
Binning is done with cumulative threshold counts in log space:
    t = max(x) - ln(sum(exp(x)));   conf >= b/15  <=>  t >= ln(b/15)
Each core computes, for b in 1..14, ct_b = #{t >= theta_b} and
cc_b = #{t >= theta_b and correct}, plus cc_0 = total correct, as
per-partition partial sums. Host diffs the cumulative counts into the
[15,2] (correct, incorrect) table.

Inputs are cast to bf16 on the host (halves HBM traffic; the resulting
~1e-3-relative jitter in conf only moves a tiny fraction of samples
across bin edges, well inside the 2e-2 gate). Each core's shard is laid
out class-blocked per chunk ([P, chunk, C, n] with n samples innermost)
so the two reduction trees (max over classes of x, sum over classes of
exp(x)) run as pairwise tensor_tensor folds over contiguous class
halves — bf16 2x_1p DVE mode — instead of per-sample accumulate ops.

Sharding: data-parallel over N across 8 cores; each 125k-sample shard is
padded to 128*992 rows. Pad rows are [1,0,...,0] with label-logit 0:
conf=e/(e+127)~0.021 < 1/15 so they never cross any threshold, and they
are never counted correct, making them invisible to the output.
"""

import numpy as np
import ml_dtypes

import concourse.bass as bass
import concourse.bacc as bacc
import concourse.tile as tile
from concourse import mybir
from concourse.bass_utils import run_bass_kernel_spmd

N_BINS = 15
C = 128
N_CORES = 8
P = 128            # SBUF partitions
NQ = 62            # samples per partition per chunk
N_CHUNKS = 16
Q = NQ * N_CHUNKS  # 992 samples per partition per core (padded)
N_PAD = P * Q      # 126976 padded samples per core

_F32 = mybir.dt.float32
_BF16 = mybir.dt.bfloat16
BF16 = ml_dtypes.bfloat16

# correct-count thresholds ride on y = 100*acc + t
_YOFF = 100.0


def _thetas() -> list[float]:
    lowers = np.linspace(0.0, 1.0, N_BINS + 1, dtype=np.float32)[:-1]
    return [float(np.log(np.float64(lowers[b]))) for b in range(1, N_BINS)]


def _build_bass() -> bass.Bass:
    nc = bacc.Bacc(None, target_bir_lowering=False)
    lg = nc.dram_tensor("logits", [P, N_CHUNKS * C * NQ], _BF16, kind="ExternalInput")
    xl = nc.dram_tensor("xl", [P, Q], _BF16, kind="ExternalInput")
    out = nc.dram_tensor("out", [P, 32], _F32, kind="ExternalOutput")

    lgv = lg[:].rearrange("p (j c q) -> p j c q", j=N_CHUNKS, c=C)
    thetas = _thetas()

    with tile.TileContext(nc) as tc:
        with (
            tc.tile_pool(name="xin", bufs=3) as xpool,
            tc.tile_pool(name="eexp", bufs=3) as epool,
            tc.tile_pool(name="mfold", bufs=2) as mpool,
            tc.tile_pool(name="sfold", bufs=2) as spool,
            tc.tile_pool(name="wide", bufs=1) as wide,
        ):
            mx = wide.tile([P, Q], _BF16)    # per-sample max(x)
            sr = wide.tile([P, Q], _F32)     # per-sample sum(exp(x))
            xlt = wide.tile([P, Q], _BF16)   # per-sample x[label]
            ls = wide.tile([P, Q], _F32)     # ln(sum exp)
            tt_t = wide.tile([P, Q], _F32)   # t = max - ln(sum exp)
            acc = wide.tile([P, Q], _BF16)   # correctness 0/1
            yy = wide.tile([P, Q], _F32)     # 100*acc + t
            junk = wide.tile([P, Q], _BF16)  # threshold scratch (bf16 ops)
            junk2 = wide.tile([P, Q], _F32)  # threshold scratch (f32 ops)
            out_t = wide.tile([P, 32], _F32)

            nc.vector.memset(out_t[:], 0.0)
            nc.sync.dma_start(out=xlt[:], in_=xl[:])

            for j in range(N_CHUNKS):
                jw = slice(j * NQ, (j + 1) * NQ)
                xt = xpool.tile([P, C, NQ], _BF16)
                nc.sync.dma_start(out=xt[:], in_=lgv[:, j])
                et = epool.tile([P, C, NQ], _BF16)
                nc.scalar.activation(
                    out=et[:], in_=xt[:], func=mybir.ActivationFunctionType.Exp
                )
                # class-axis max fold: 128 -> 64 -> 32 -> 16, then reduce
                ms = mpool.tile([P, 64, NQ], _BF16)
                nc.vector.tensor_tensor(
                    ms[:], xt[:, 0:64, :], xt[:, 64:128, :], mybir.AluOpType.max
                )
                nc.vector.tensor_tensor(
                    ms[:, 0:32, :], ms[:, 0:32, :], ms[:, 32:64, :],
                    mybir.AluOpType.max,
                )
                nc.vector.tensor_tensor(
                    ms[:, 0:16, :], ms[:, 0:16, :], ms[:, 16:32, :],
                    mybir.AluOpType.max,
                )
                nc.vector.tensor_reduce(
                    out=mx[:, jw],
                    in_=ms[:, 0:16, :].rearrange("p c q -> p q c"),
                    axis=mybir.AxisListType.X,
                    op=mybir.AluOpType.max,
                )
                # class-axis sum fold on exp tile
                ss = spool.tile([P, 64, NQ], _BF16)
                nc.vector.tensor_tensor(
                    ss[:], et[:, 0:64, :], et[:, 64:128, :], mybir.AluOpType.add
                )
                nc.vector.tensor_tensor(
                    ss[:, 0:32, :], ss[:, 0:32, :], ss[:, 32:64, :],
                    mybir.AluOpType.add,
                )
                nc.vector.tensor_tensor(
                    ss[:, 0:16, :], ss[:, 0:16, :], ss[:, 16:32, :],
                    mybir.AluOpType.add,
                )
                nc.vector.tensor_reduce(
                    out=sr[:, jw],
                    in_=ss[:, 0:16, :].rearrange("p c q -> p q c"),
                    axis=mybir.AxisListType.X,
                    op=mybir.AluOpType.add,
                )

            # ---- per-sample epilogue on [128, Q] wide tiles ----
            nc.scalar.activation(
                out=ls[:], in_=sr[:], func=mybir.ActivationFunctionType.Ln
            )
            nc.vector.tensor_tensor(
                tt_t[:], mx[:], ls[:], mybir.AluOpType.subtract
            )
            nc.vector.tensor_tensor(acc[:], xlt[:], mx[:], mybir.AluOpType.is_equal)
            # y = 100*acc + t: thresholds at 100+theta count correct-and-binned
            nc.vector.scalar_tensor_tensor(
                out=yy[:], in0=acc[:], scalar=_YOFF, in1=tt_t[:],
                op0=mybir.AluOpType.mult, op1=mybir.AluOpType.add,
            )
            # col 16: total correct (every correct sample has y >= 50)
            nc.vector.tensor_scalar(
                junk2[:], yy[:], _YOFF / 2, None, mybir.AluOpType.is_ge,
                op1=mybir.AluOpType.add, accum_out=out_t[:, 16:17],
            )
            for b in range(1, N_BINS):
                nc.vector.tensor_scalar(
                    junk2[:], tt_t[:], thetas[b - 1], None, mybir.AluOpType.is_ge,
                    op1=mybir.AluOpType.add, accum_out=out_t[:, b : b + 1],
                )
                nc.vector.tensor_scalar(
                    junk2[:], yy[:], _YOFF + thetas[b - 1], None,
                    mybir.AluOpType.is_ge,
                    op1=mybir.AluOpType.add, accum_out=out_t[:, 16 + b : 17 + b],
                )
            nc.sync.dma_start(out=out[:], in_=out_t[:])
    nc.compile()
    return nc


_NC_CACHE = None


def _get_nc() -> bass.Bass:
    global _NC_CACHE
    if _NC_CACHE is None:
        _NC_CACHE = _build_bass()
    return _NC_CACHE


def make_in_maps(logits: np.ndarray, labels: np.ndarray):
    """Shard + pad + bf16-cast + class-block-transpose full inputs."""
    n = logits.shape[0]
    n_per = n // N_CORES
    assert n_per * N_CORES == n and n_per <= N_PAD
    lb = logits.astype(BF16)
    idx = np.arange(n, dtype=np.int64)
    xl_full = lb[idx, labels.astype(np.int64)]

    pad_row = np.zeros(C, BF16)
    pad_row[0] = 1.0
    in_maps = []
    for s in range(N_CORES):
        lo = s * n_per
        shard = np.empty((N_PAD, C), BF16)
        shard[:n_per] = lb[lo : lo + n_per]
        shard[n_per:] = pad_row  # conf ~0.021 (bin 0), never correct
        # sample (p, q) at row p*Q + q; chunk-block and put classes outer
        tr = np.ascontiguousarray(
            shard.reshape(P, N_CHUNKS, NQ, C).transpose(0, 1, 3, 2)
        )
        xls = np.zeros(N_PAD, BF16)
        xls[:n_per] = xl_full[lo : lo + n_per]
        in_maps.append(
            {"logits": tr.reshape(P, N_CHUNKS * C * NQ), "xl": xls.reshape(P, Q)}
        )
    return in_maps


def combine_outputs(results, n: int) -> np.ndarray:
    """Fold per-core [128,32] partials into the [15,2] bin_stats table."""
    ct = np.zeros(N_BINS + 1, np.float64)  # cumulative totals, index b
    cc = np.zeros(N_BINS + 1, np.float64)  # cumulative corrects
    ct[0] = float(n)
    for r in results:
        o = np.asarray(r["out"], np.float64)
        colsum = o.sum(axis=0)
        ct[1:N_BINS] += colsum[1:N_BINS]
        cc[0] += colsum[16]
        cc[1:N_BINS] += colsum[17 : 16 + N_BINS]
    total = ct[:-1] - ct[1:]
    correct = cc[:-1] - cc[1:]
    return np.stack([correct, total - correct], axis=1).astype(np.float32)


def kernel(logits, labels) -> np.ndarray:
    logits = np.asarray(logits, dtype=np.float32)
    labels = np.asarray(labels)
    n = logits.shape[0]
    in_maps = make_in_maps(logits, labels)
    res = run_bass_kernel_spmd(_get_nc(), in_maps, core_ids=list(range(N_CORES)))
    return combine_outputs(res.results, n)
